# revision 1
# baseline (speedup 1.0000x reference)
"""CQAttention Trainium2 kernel — fp16, software-pipelined.

Math (per batch b, D=128, Lc=1024, Lq=128):
    Ct = C[b].T  (Lc,D);  Qt = Q[b].T  (Lq,D)
    S[c,q] = (Ct[c]*w_m).Qt[q] + Ct[c].w_c + Qt[q].w_q
    S1 = softmax_q(S + qbias), S2 = softmax_c(S + cbias)
    A  = S1 @ Qt
    Bt = (S1 @ S2.T) @ Ct = S1 @ (S2.T @ Ct)       # W2 := S2.T@Ct is 128x128
    out[b] = concat([Ct, A, Ct*A, Ct*Bt], axis=1).T  # [4D, Lc]

Device computes blocks 1..3 (A, Ct*A, Ct*Bt) in fp16; block 0 is exactly
the input C[b], assembled on the host in fp32.  The rank-1 bias terms
s_q = Qt.w_q + qbias (fp32 input) and s_c = Ct.w_c + cbias (fp16 cols)
are tiny host matvecs.

Softmax factorization (shift-free; logits are O(1)):
    S1[c,q] = exp(s_m + s_q)[c,q] / z1[c],  z1[c] = sum_q exp(s_m+s_q)
    S2[c,q] = exp(s_m + s_c)[c,q] / z2[q],  z2[q] = sum_c exp(s_m+s_c)
exp(s_c) is applied multiplicatively on the w2 GEMM's moving operand
(rhsB = [CT|1] * esc), so the e2 exps run biasless as 4x256-col acts.

Pipeline: batch b's work is split into stages emitted two iterations
apart so every engine's in-order queue interleaves consecutive batches:
  iter k:  st/e1/z1/at/s0 of batch k-1,  w2/bt/out of batch k-2,
           load of batch k.
The output DMA is issued from SP after the loads so it never
head-of-line blocks the input stream.
"""

import warnings

warnings.filterwarnings("ignore")

import numpy as np

B, D, LC, LQ = 64, 128, 1024, 128
NT = 8  # c-tiles per batch
NCORES = 8
NB = B // NCORES  # batches per core
NEG16 = -30000.0  # fp16-representable "minus infinity" for mask biases
NIN = 2 * LC + 2 * LQ + NT  # row: cb | qww | esc8 | ctb | qtb
CTB0 = LC + LQ + NT
QTB0 = CTB0 + LC

CFG = {
    "pipe": 2,        # stage skew
    "hostT": True,    # host-side transposed C/Q (no XBAR)
    "v12": True,      # host qww/esc, halved chain ops
}

_CACHE = {}


def _build_nc(reps=1):
    import concourse.bass as bass
    import concourse.mybir as mybir
    import concourse.tile as tile
    from concourse import bacc

    F32 = mybir.dt.float32
    F16 = mybir.dt.float16
    AF = mybir.ActivationFunctionType

    nc = bacc.Bacc("TRN2", target_bir_lowering=False, debug=False,
                   num_devices=NCORES)

    In16 = nc.dram_tensor("In16", [NB, D, NIN], F16, kind="ExternalInput")
    SQ32 = nc.dram_tensor("SQ32", [NB, D, 1], F32, kind="ExternalInput")
    Out = nc.dram_tensor("Out", [NB, D, 3, LC], F16, kind="ExternalOutput")

    with tile.TileContext(nc) as tc:
        with tc.tile_pool(name="const", bufs=1) as constp, \
             tc.tile_pool(name="io", bufs=4) as iop, \
             tc.tile_pool(name="sb", bufs=4) as sb, \
             tc.tile_pool(name="sm", bufs=4) as sm, \
             tc.tile_pool(name="ps_big", bufs=3, space="PSUM") as ps_big, \
             tc.tile_pool(name="ps_s0", bufs=2, space="PSUM") as ps_s0:

            # ---- constants ----
            ones16 = constp.tile([D, D], F16)
            nc.gpsimd.memset(ones16[:], 1.0)

            def batch_phases(b):
                st = {}

                def ph_load():
                    inb = iop.tile([D, NIN], F16, tag="inb", name=f"inb{b}")
                    st["inb"] = inb
                    nc.sync.dma_start(inb[:], In16[b])
                    sq32 = sm.tile([D, 1], F32, tag="sq32", name=f"sq32{b}")
                    st["sq32"] = sq32
                    nc.sync.dma_start(sq32[:], SQ32[b])

                def ph_st():
                    inb = st["inb"]
                    cb = inb[:, 0:LC]
                    qww = inb[:, LC:LC + LQ]
                    st["qww"] = qww
                    esc = inb[:, LC + LQ:CTB0]
                    rhsB = sb.tile([D, NT, 132], F16, tag="rhsB",
                                   name=f"rhsB{b}")
                    st["rhsB"] = rhsB
                    ctb = inb[:, CTB0:CTB0 + LC].rearrange(
                        "p (t j) -> p t j", t=NT)
                    nc.gpsimd.tensor_mul(
                        rhsB[:, :, 0:128], ctb,
                        esc[:, :, None].broadcast_to((D, NT, D)))
                    nc.gpsimd.tensor_copy(rhsB[:, :, 128:129],
                                          esc[:, :, None])
                    p_st = ps_big.tile([D, LC], F32, tag="big", name=f"pst{b}")
                    st["p_st"] = p_st
                    for h in range(2):
                        nc.tensor.matmul(p_st[:, 512 * h:512 * (h + 1)],
                                         qww[:],
                                         cb[:, 512 * h:512 * (h + 1)],
                                         start=True, stop=True)

                def ph_e1():
                    p_st = st["p_st"]
                    e1 = sb.tile([D, LC], F16, tag="e1", name=f"e1_{b}")
                    st["e1"] = e1
                    for h in range(2):
                        nc.scalar.activation(e1[:, 512 * h:512 * (h + 1)],
                                             p_st[:, 512 * h:512 * (h + 1)],
                                             AF.Exp, bias=st["sq32"][:])

                def ph_z1at():
                    e1 = st["e1"]
                    qt = st["inb"][:, QTB0:QTB0 + LQ]
                    inb = st["inb"]
                    cb = inb[:, 0:LC]
                    p_z1 = ps_big.tile([D, LC], F32, tag="big", name=f"pz1{b}")
                    for h in range(2):
                        nc.tensor.matmul(p_z1[:, 512 * h:512 * (h + 1)],
                                         ones16[:],
                                         e1[:, 512 * h:512 * (h + 1)],
                                         start=True, stop=True)
                    r1 = sb.tile([D, LC], F32, tag="r1", name=f"r1_{b}")
                    st["r1"] = r1
                    for h in range(2):
                        nc.vector.reciprocal_approx_fast(
                            r1[:, 512 * h:512 * (h + 1)],
                            p_z1[:, 512 * h:512 * (h + 1)])
                    p_at = ps_big.tile([D, LC], F32, tag="big", name=f"pat{b}")
                    for h in range(2):
                        nc.tensor.matmul(p_at[:, 512 * h:512 * (h + 1)], qt,
                                         e1[:, 512 * h:512 * (h + 1)],
                                         start=True, stop=True)
                    ob = iop.tile([D, 3, LC], F16, tag="ob", name=f"ob{b}")
                    st["ob"] = ob
                    for h in range(2):
                        nc.vector.tensor_mul(
                            ob[:, 0, 512 * h:512 * (h + 1)],
                            p_at[:, 512 * h:512 * (h + 1)],
                            r1[:, 512 * h:512 * (h + 1)])
                    nc.gpsimd.tensor_mul(ob[:, 1, :], ob[:, 0, :], cb[:])

                def ph_s0():
                    inb, qww = st["inb"], st["qww"]
                    cb = inb[:, 0:LC]
                    e2 = sb.tile([D, NT, D], F16, tag="e2", name=f"e2_{b}")
                    st["e2"] = e2
                    for g in range(4):
                        p_s0 = ps_s0.tile([D, 2, 256], F32, tag="s0",
                                          name=f"ps0{b}_{g}")
                        for j in range(2):
                            ct = 2 * g + j
                            nc.tensor.matmul(p_s0[:, j, 0:128],
                                             cb[:, ct * D:(ct + 1) * D],
                                             qww[:],
                                             start=True, stop=True)
                        nc.scalar.activation(e2[:, 2 * g:2 * g + 2, :],
                                             p_s0[:, :, 0:128], AF.Exp)

                def ph_w2():
                    e2, rhsB = st["e2"], st["rhsB"]
                    p_w2 = ps_s0.tile([D, 2, 256], F32, tag="s0",
                                      name=f"pw2{b}")
                    for ct in range(NT):
                        nc.tensor.matmul(p_w2[:, 0, 0:129], e2[:, ct, :],
                                         rhsB[:, ct, 0:129],
                                         start=(ct == 0), stop=(ct == NT - 1))
                    r2 = sm.tile([D, 1], F32, tag="r2", name=f"r2_{b}")
                    nc.vector.reciprocal(r2[:], p_w2[:, 0, 128:129])
                    w2 = sb.tile([D, D], F16, tag="w2sb", name=f"w2_{b}")
                    st["w2"] = w2
                    nc.scalar.activation(w2[:], p_w2[:, 0, 0:128], AF.Copy,
                                         scale=r2[:])

                def ph_bt():
                    e1, w2, r1 = st["e1"], st["w2"], st["r1"]
                    inb, ob = st["inb"], st["ob"]
                    cb = inb[:, 0:LC]
                    p_bt = ps_big.tile([D, LC], F32, tag="big", name=f"pbt{b}")
                    for h in range(2):
                        nc.tensor.matmul(p_bt[:, 512 * h:512 * (h + 1)], w2[:],
                                         e1[:, 512 * h:512 * (h + 1)],
                                         start=True, stop=True)
                    t3 = sb.tile([D, LC], F16, tag="t3", name=f"t3_{b}")
                    for h in range(2):
                        nc.vector.tensor_mul(t3[:, 512 * h:512 * (h + 1)],
                                             p_bt[:, 512 * h:512 * (h + 1)],
                                             r1[:, 512 * h:512 * (h + 1)])
                    nc.gpsimd.tensor_mul(ob[:, 2, :], t3[:], cb[:])

                def emit_out():
                    nc.sync.dma_start(Out[b], st["ob"][:])

                st["phases"] = dict(load=ph_load, st=ph_st, e1=ph_e1,
                                    z1at=ph_z1at, s0=ph_s0, w2=ph_w2,
                                    bt=ph_bt, out=emit_out)
                return st

            order = [bb for _ in range(reps) for bb in range(NB)]
            N = len(order)
            P = {}
            for k in range(N + 2):
                if 0 <= k - 1 < N:
                    P[k - 1]["phases"]["st"]()
                if 0 <= k - 2 < N:
                    P[k - 2]["phases"]["w2"]()
                if k < N:
                    P[k] = batch_phases(order[k])
                    P[k]["phases"]["load"]()
                if 0 <= k - 2 < N:
                    P[k - 2]["phases"]["bt"]()
                    P[k - 2]["phases"]["out"]()
                if 0 <= k - 1 < N:
                    P[k - 1]["phases"]["e1"]()
                    P[k - 1]["phases"]["z1at"]()
                    P[k - 1]["phases"]["s0"]()

    nc.compile()
    return nc


def _prep_inmaps(C, Q, cmask, qmask, w):
    C64 = np.asarray(C, np.float64)
    Q64 = np.asarray(Q, np.float64)
    w64 = np.asarray(w, np.float64)
    wq, wc = w64[:D], w64[D:2 * D]
    C16 = C64.astype(np.float16)                          # [B, D, LC]
    Q16 = Q64.astype(np.float16)                          # [B, D, LQ]
    qbias = (1.0 - np.asarray(qmask, np.float64)) * NEG16
    cbias = (1.0 - np.asarray(cmask, np.float64)) * NEG16
    sq32 = np.ascontiguousarray(
        (np.einsum("bdq,d->bq", Q64, wq) + qbias)
        .astype(np.float32)[:, :, None])                  # [B, D(q), 1]
    scb = np.einsum("bdc,d->bc", C64, wc) + cbias
    esc8 = np.ascontiguousarray(
        np.exp(scb).reshape(B, NT, D).transpose(0, 2, 1)
        .astype(np.float16))                              # [B, D, NT]
    qww16 = (Q64 * w64[2 * D:][None, :, None]).astype(np.float16)
    ctb = np.ascontiguousarray(
        C16.reshape(B, D, NT, D).transpose(0, 3, 2, 1)
        .reshape(B, D, LC))                               # [b, p, (t, j)]
    qtb = np.ascontiguousarray(Q16.transpose(0, 2, 1))    # [b, q, d]
    in16 = np.concatenate([C16, qww16, esc8, ctb, qtb], axis=2)
    in16 = np.ascontiguousarray(in16)
    in_maps = []
    for k in range(NCORES):
        s = slice(k * NB, (k + 1) * NB)
        in_maps.append({"In16": in16[s], "SQ32": sq32[s]})
    return in_maps


def _run(C, Q, cmask, qmask, w, trace=False):
    from concourse.bass_utils import run_bass_kernel_spmd

    key = (tuple(sorted(CFG.items())), 1)
    if key not in _CACHE:
        _CACHE[key] = _build_nc()
    nc = _CACHE[key]
    in_maps = _prep_inmaps(C, Q, cmask, qmask, w)
    res = run_bass_kernel_spmd(nc, in_maps, core_ids=list(range(NCORES)),
                               trace=trace)
    dev = np.concatenate([res.results[k]["Out"] for k in range(NCORES)],
                         axis=0)                          # [B, D, 3, LC] fp16
    return dev, res


def _assemble(dev, C):
    """dev: [n, D, 3, LC] fp16 device blocks; C: [n, D, LC] input."""
    n = dev.shape[0]
    out = np.empty((n, 4 * D, LC), np.float32)
    out[:, 0:D, :] = np.asarray(C, np.float32)            # block 0 exact
    out[:, D:4 * D, :] = dev.transpose(0, 2, 1, 3).reshape(
        n, 3 * D, LC).astype(np.float32)
    return out


def kernel(C, Q, cmask, qmask, w):
    dev, _ = _run(C, Q, cmask, qmask, w, trace=False)
    return _assemble(dev, C)



# revision 3
# speedup vs baseline: 1.4218x; 1.4218x over previous
"""CQAttention Trainium2 kernel v2 — raw A/Bt export, host normalization.

Math (per batch b, D=128, Lc=1024, Lq=128):
    S[c,q] = (Ct[c]*w_m).Qt[q] + Ct[c].w_c + Qt[q].w_q
    S1 = softmax_q(S + qbias), S2 = softmax_c(S + cbias)
    A  = S1 @ Qt;  Bt = S1 @ (S2.T @ Ct) = S1 @ W2
    out[b] = concat([Ct, A, Ct*A, Ct*Bt], axis=1).T

Device computes, per batch, in [c,d] output layout:
    e1[q,c]   = exp(s_m^T + s_q - ln16)            (fp16, ACT, bias=sq32)
    e2[c,q]   = exp(s_m)  (8 c-tiles)              (fp16, ACT)
    W2[q,d]   = (e2^T @ [Ct*esc | esc]) / z2       (8 accum MMs + recip + act)
    Araw[c,d] = e1_t^T @ Qt      (8 tiles)         = (S1-unnorm/16) @ Qt
    Btraw[c,d]= e1_t^T @ W2
    z[c]      = e1_t^T @ 1                         = z1/16
Exports fp16 [c, t, {Araw(128) | Btraw(128) | z(1)}]; the host divides by z
(scales cancel), transposes to [d, c], and assembles
[Ct, A, Ct*A, Ct*Bt] in fp32.  Rank-1 terms s_q, s_c and exp(s_c) are host
matvecs; rhsB = [Ct*esc | esc] is host-prepped in [c,d] layout.

PSUM (8 banks): pool p1 bufs=4 x 1 bank rotating 5 tiles/batch
(t2_0, t2_1 = S-tiles, t1_0, t1_1 = S^T halves, t3 = W2-accum + z cols);
pool ab bufs=2 x 2 banks (A/Bt halves, 4 c-tiles each).

Pipeline (iter k): s0/e2/st/e1/w2 of batch k-1, ab/copies/out of batch k-2,
load of batch k.  Input DMA on SP, output DMA on Pool (swdge) so the two
streams ride different queues.
"""

import warnings

warnings.filterwarnings("ignore")

import numpy as np

B, D, LC, LQ = 64, 128, 1024, 128
NT = 8  # c-tiles per batch
NCORES = 8
NB = B // NCORES  # batches per core
NEG16 = -30000.0  # fp16-representable "minus infinity" for mask biases
NIN = LC + LQ + NT * 129 + LQ  # row: cb | qww | rhsB(8x129) | qt
RB0 = LC + LQ
QT0 = RB0 + NT * 129
NOUT = NT * 257
LN16 = float(np.log(16.0))

CFG = {"v": 2}

_CACHE = {}


def _build_nc(reps=1):
    import concourse.bass as bass
    import concourse.mybir as mybir
    import concourse.tile as tile
    from concourse import bacc

    F32 = mybir.dt.float32
    F16 = mybir.dt.float16
    AF = mybir.ActivationFunctionType

    nc = bacc.Bacc("TRN2", target_bir_lowering=False, debug=False,
                   num_devices=NCORES)

    In16 = nc.dram_tensor("In16", [NB, D, NIN], F16, kind="ExternalInput")
    SQ32 = nc.dram_tensor("SQ32", [NB, D, 1], F32, kind="ExternalInput")
    Out = nc.dram_tensor("Out", [NB, D, NT, 257], F16, kind="ExternalOutput")

    with tile.TileContext(nc) as tc:
        with tc.tile_pool(name="const", bufs=1) as constp, \
             tc.tile_pool(name="io", bufs=4) as iop, \
             tc.tile_pool(name="sb", bufs=4) as sb, \
             tc.tile_pool(name="sm", bufs=4) as sm, \
             tc.tile_pool(name="p1", bufs=4, space="PSUM") as p1, \
             tc.tile_pool(name="ab", bufs=2, space="PSUM") as abp:

            ones1 = constp.tile([D, 1], F16)
            nc.gpsimd.memset(ones1[:], 1.0)

            def batch_phases(b):
                st = {}

                def ph_load():
                    inb = iop.tile([D, NIN], F16, tag="inb", name=f"inb{b}")
                    st["inb"] = inb
                    nc.sync.dma_start(inb[:], In16[b])
                    sq = sm.tile([D, 1], F32, tag="sq", name=f"sq{b}")
                    st["sq"] = sq
                    nc.sync.dma_start(sq[:], SQ32[b])

                def ph_s0():
                    inb = st["inb"]
                    cb = inb[:, 0:LC]
                    qww = inb[:, LC:LC + LQ]
                    for h in range(2):
                        t2 = p1.tile([D, 4, D], F32, tag="p1",
                                     name=f"t2_{b}_{h}")
                        st[f"t2{h}"] = t2
                        for i in range(4):
                            ct = 4 * h + i
                            nc.tensor.matmul(t2[:, i, :],
                                             cb[:, ct * D:(ct + 1) * D],
                                             qww[:], start=True, stop=True)

                def ph_e2():
                    e2 = sb.tile([D, NT, D], F16, tag="e2", name=f"e2_{b}")
                    st["e2"] = e2
                    for h in range(2):
                        nc.scalar.activation(e2[:, 4 * h:4 * h + 4, :],
                                             st[f"t2{h}"][:], AF.Exp)

                def ph_st():
                    inb = st["inb"]
                    cb = inb[:, 0:LC]
                    qww = inb[:, LC:LC + LQ]
                    for h in range(2):
                        t1 = p1.tile([D, 512], F32, tag="p1",
                                     name=f"t1_{b}_{h}")
                        st[f"t1{h}"] = t1
                        nc.tensor.matmul(t1[:], qww[:],
                                         cb[:, 512 * h:512 * (h + 1)],
                                         start=True, stop=True)

                def ph_e1():
                    e1 = sb.tile([D, LC], F16, tag="e1", name=f"e1_{b}")
                    st["e1"] = e1
                    for h in range(2):
                        nc.scalar.activation(e1[:, 512 * h:512 * (h + 1)],
                                             st[f"t1{h}"][:], AF.Exp,
                                             bias=st["sq"][:])

                def ph_w2():
                    inb, e2 = st["inb"], st["e2"]
                    rb = inb[:, RB0:QT0].rearrange("p (t j) -> p t j", t=NT)
                    t3 = p1.tile([D, 512], F32, tag="p1", name=f"t3_{b}")
                    st["t3"] = t3
                    for t in range(NT):
                        nc.tensor.matmul(t3[:, 0:129], e2[:, t, :],
                                         rb[:, t, :],
                                         start=(t == 0), stop=(t == NT - 1))
                    r2 = sm.tile([D, 1], F32, tag="r2", name=f"r2_{b}")
                    nc.vector.reciprocal(r2[:], t3[:, 128:129])
                    w2 = sb.tile([D, D], F16, tag="w2", name=f"w2_{b}")
                    st["w2"] = w2
                    nc.scalar.activation(w2[:], t3[:, 0:128], AF.Copy,
                                         scale=r2[:])

                def ph_ab(h):
                    inb, e1, w2, t3 = st["inb"], st["e1"], st["w2"], st["t3"]
                    qt = inb[:, QT0:QT0 + LQ]
                    ab = abp.tile([D, 4, 256], F32, tag="ab",
                                  name=f"ab{b}_{h}")
                    st[f"ab{h}"] = ab
                    for i in range(4):
                        t = 4 * h + i
                        lhsT = e1[:, t * D:(t + 1) * D]
                        nc.tensor.matmul(ab[:, i, 0:D], lhsT, qt[:],
                                         start=True, stop=True)
                        nc.tensor.matmul(ab[:, i, D:2 * D], lhsT, w2[:],
                                         start=True, stop=True)
                        nc.tensor.matmul(t3[:, 384 + t:385 + t], lhsT,
                                         ones1[:], start=True, stop=True)

                def ph_ob_alloc():
                    ob = iop.tile([D, NT, 257], F16, tag="ob", name=f"ob{b}")
                    st["ob"] = ob

                def ph_zfin():
                    dst = st["ob"][:, :, 256:257].rearrange("p t o -> p (t o)")
                    nc.vector.tensor_copy(dst, st["t3"][:, 384:392])

                def ph_obh(h):
                    nc.vector.tensor_copy(
                        st["ob"][:, 4 * h:4 * h + 4, 0:256], st[f"ab{h}"][:])

                def ph_out():
                    nc.sync.dma_start(Out[b], st["ob"][:])

                st["phases"] = dict(
                    load=ph_load, s0=ph_s0, e2=ph_e2, st=ph_st, e1=ph_e1,
                    w2=ph_w2, ab0=lambda: ph_ab(0), ab1=lambda: ph_ab(1),
                    oba=ph_ob_alloc, zfin=ph_zfin,
                    ob0=lambda: ph_obh(0), ob1=lambda: ph_obh(1), out=ph_out)
                return st

            order = [bb for _ in range(reps) for bb in range(NB)]
            N = len(order)
            P = {}
            for k in range(N + 2):
                if 0 <= k - 1 < N:
                    P[k - 1]["phases"]["s0"]()
                    P[k - 1]["phases"]["e2"]()
                if 0 <= k - 2 < N:
                    P[k - 2]["phases"]["oba"]()
                    P[k - 2]["phases"]["ab0"]()
                    P[k - 2]["phases"]["ab1"]()
                    P[k - 2]["phases"]["zfin"]()
                    P[k - 2]["phases"]["ob0"]()
                if 0 <= k - 1 < N:
                    P[k - 1]["phases"]["st"]()
                    P[k - 1]["phases"]["e1"]()
                    P[k - 1]["phases"]["w2"]()
                if 0 <= k - 2 < N:
                    P[k - 2]["phases"]["ob1"]()
                    P[k - 2]["phases"]["out"]()
                if k < N:
                    P[k] = batch_phases(order[k])
                    P[k]["phases"]["load"]()

    nc.compile()
    return nc


def _prep_inmaps(C, Q, cmask, qmask, w):
    C64 = np.asarray(C, np.float64)
    Q64 = np.asarray(Q, np.float64)
    w64 = np.asarray(w, np.float64)
    wq, wc, wm = w64[:D], w64[D:2 * D], w64[2 * D:]
    C16 = C64.astype(np.float16)                          # [B, D, LC]
    qww16 = (Q64 * wm[None, :, None]).astype(np.float16)  # [B, D, LQ]
    qbias = (1.0 - np.asarray(qmask, np.float64)) * NEG16
    cbias = (1.0 - np.asarray(cmask, np.float64)) * NEG16
    sq32 = np.ascontiguousarray(
        (np.einsum("bdq,d->bq", Q64, wq) + qbias - LN16)
        .astype(np.float32)[:, :, None])                  # [B, q, 1]
    scb = np.einsum("bdc,d->bc", C64, wc) + cbias         # [B, LC]
    esc = np.exp(scb)                                     # [B, LC]
    ctesc = C64 * esc[:, None, :]                         # [B, D, LC]
    rb = np.empty((B, D, NT, 129), np.float16)
    rb[..., 0:128] = ctesc.reshape(B, D, NT, D).transpose(0, 3, 2, 1)
    rb[..., 128] = esc.reshape(B, NT, D).transpose(0, 2, 1)
    qtb = Q64.astype(np.float16).transpose(0, 2, 1)       # [B, q, d]
    in16 = np.concatenate(
        [C16, qww16, rb.reshape(B, D, NT * 129), qtb], axis=2)
    in16 = np.ascontiguousarray(in16)
    in_maps = []
    for k in range(NCORES):
        s = slice(k * NB, (k + 1) * NB)
        in_maps.append({"In16": in16[s], "SQ32": sq32[s]})
    return in_maps


def _run(C, Q, cmask, qmask, w, trace=False):
    from concourse.bass_utils import run_bass_kernel_spmd

    key = (tuple(sorted(CFG.items())), 1)
    if key not in _CACHE:
        _CACHE[key] = _build_nc()
    nc = _CACHE[key]
    in_maps = _prep_inmaps(C, Q, cmask, qmask, w)
    res = run_bass_kernel_spmd(nc, in_maps, core_ids=list(range(NCORES)),
                               trace=trace)
    dev = np.concatenate([res.results[k]["Out"] for k in range(NCORES)],
                         axis=0)                          # [B, D, NT, 257]
    return dev, res


def _assemble(dev, C):
    """dev: [n, 128(j), 8(t), 257] fp16; C: [n, D, LC] input."""
    n = dev.shape[0]
    dv = np.asarray(dev, np.float32)
    r = 1.0 / dv[..., 256]                                # [n, j, t]
    A = dv[..., 0:128] * r[..., None]                     # [n, j, t, d]
    Bt = dv[..., 128:256] * r[..., None]
    At = np.ascontiguousarray(A.transpose(0, 3, 2, 1)).reshape(n, D, LC)
    Btt = np.ascontiguousarray(Bt.transpose(0, 3, 2, 1)).reshape(n, D, LC)
    Cf = np.asarray(C, np.float32)
    out = np.empty((n, 4 * D, LC), np.float32)
    out[:, 0:D] = Cf
    out[:, D:2 * D] = At
    out[:, 2 * D:3 * D] = Cf * At
    out[:, 3 * D:4 * D] = Cf * Btt
    return out


def kernel(C, Q, cmask, qmask, w):
    dev, _ = _run(C, Q, cmask, qmask, w, trace=False)
    return _assemble(dev, C)


# revision 4
# speedup vs baseline: 1.5215x; 1.0701x over previous
"""CQAttention Trainium2 kernel v3 — [d,c] A/Bt, raw export, host norm.

Device per batch (D=128, Lc=1024, Lq=128):
    e1[q,c]    = exp(s_m^T + s_q - ln16)     ACT, bias = sq32 (bitcast cols)
    e2[c,q]    = exp(s_m)   (8 c-tiles)      ACT
    W2[q,d]    = (e2^T @ [Ct*esc | esc])/z2  8 accum MMs + recip + act
    At[d,c]    = qt^T @ e1                   2 MMs N=512   (= (S1raw/16 @ Qt)^T)
    Bt[d,c]    = w2^T @ e1                   2 MMs N=512
    z[c]=z1/16 = e1_t^T @ 1                  8 tiny MMs (e1-tile stationaries)
Exports fp16 [d, {At 1024 | Bt 1024 | z 8}]; host divides by z (scales
cancel) and assembles [Ct, A, Ct*A, Ct*Bt] in fp32.

PSUM (8 banks): p1 bufs=4 x 1 bank rotating 5 tiles/batch (t2a,t2b = S-tile
quads, t1a,t1b = S^T halves, t3 = W2-accum + z cols); at/bt pools bufs=1 x
2 banks each.
"""

import warnings

warnings.filterwarnings("ignore")

import numpy as np

B, D, LC, LQ = 64, 128, 1024, 128
NT = 8
NCORES = 8
NB = B // NCORES
NEG16 = -30000.0
NIN = LC + LQ + NT * 129 + LQ + 1  # cb | qww | rhsB | qt | sq16
RB0 = LC + LQ
QT0 = RB0 + NT * 129
SQ0 = QT0 + LQ
NOUT = 2 * LC + NT  # At | Bt | z
LN16 = float(np.log(16.0))

CFG = {"v": 3}

_CACHE = {}


def _build_nc(reps=1):
    import concourse.bass as bass
    import concourse.mybir as mybir
    import concourse.tile as tile
    from concourse import bacc

    F32 = mybir.dt.float32
    F16 = mybir.dt.float16
    AF = mybir.ActivationFunctionType

    nc = bacc.Bacc("TRN2", target_bir_lowering=False, debug=False,
                   num_devices=NCORES)

    In16 = nc.dram_tensor("In16", [NB, D, NIN], F16, kind="ExternalInput")
    Out = nc.dram_tensor("Out", [NB, D, NOUT], F16, kind="ExternalOutput")

    with tile.TileContext(nc) as tc:
        with tc.tile_pool(name="const", bufs=1) as constp, \
             tc.tile_pool(name="io", bufs=4) as iop, \
             tc.tile_pool(name="sb", bufs=4) as sb, \
             tc.tile_pool(name="sm", bufs=4) as sm, \
             tc.tile_pool(name="p1", bufs=4, space="PSUM") as p1, \
             tc.tile_pool(name="at", bufs=1, space="PSUM") as atp, \
             tc.tile_pool(name="bt", bufs=1, space="PSUM") as btp:

            ones1 = constp.tile([D, 1], F16)
            nc.gpsimd.memset(ones1[:], 1.0)

            def batch_phases(b):
                st = {}

                def ph_load():
                    inb = iop.tile([D, NIN], F16, tag="inb", name=f"inb{b}")
                    st["inb"] = inb
                    nc.sync.dma_start(inb[:], In16[b])

                def ph_s0():
                    inb = st["inb"]
                    cb = inb[:, 0:LC]
                    qww = inb[:, LC:LC + LQ]
                    for h in range(2):
                        t2 = p1.tile([D, 4, D], F32, tag="p1",
                                     name=f"t2_{b}_{h}")
                        st[f"t2{h}"] = t2
                        for i in range(4):
                            ct = 4 * h + i
                            nc.tensor.matmul(t2[:, i, :],
                                             cb[:, ct * D:(ct + 1) * D],
                                             qww[:], start=True, stop=True)

                def ph_e2():
                    e2 = sb.tile([D, NT, D], F16, tag="e2", name=f"e2_{b}")
                    st["e2"] = e2
                    for h in range(2):
                        nc.scalar.activation(e2[:, 4 * h:4 * h + 4, :],
                                             st[f"t2{h}"][:], AF.Exp)

                def ph_st():
                    inb = st["inb"]
                    cb = inb[:, 0:LC]
                    qww = inb[:, LC:LC + LQ]
                    for h in range(2):
                        t1 = p1.tile([D, 512], F32, tag="p1",
                                     name=f"t1_{b}_{h}")
                        st[f"t1{h}"] = t1
                        nc.tensor.matmul(t1[:], qww[:],
                                         cb[:, 512 * h:512 * (h + 1)],
                                         start=True, stop=True)

                def ph_e1():
                    inb = st["inb"]
                    sq = inb[:, SQ0:SQ0 + 1]
                    e1 = sb.tile([D, LC], F16, tag="e1", name=f"e1_{b}")
                    st["e1"] = e1
                    for h in range(2):
                        nc.scalar.activation(e1[:, 512 * h:512 * (h + 1)],
                                             st[f"t1{h}"][:], AF.Exp,
                                             bias=sq)

                def ph_w2():
                    inb, e2 = st["inb"], st["e2"]
                    rb = inb[:, RB0:QT0].rearrange("p (t j) -> p t j", t=NT)
                    t3 = p1.tile([D, 512], F32, tag="p1", name=f"t3_{b}")
                    st["t3"] = t3
                    for t in range(NT):
                        nc.tensor.matmul(t3[:, 0:129], e2[:, t, :],
                                         rb[:, t, :],
                                         start=(t == 0), stop=(t == NT - 1))
                    r2 = sm.tile([D, 1], F32, tag="r2", name=f"r2_{b}")
                    nc.vector.reciprocal(r2[:], t3[:, 128:129])
                    w2 = sb.tile([D, D], F16, tag="w2", name=f"w2_{b}")
                    st["w2"] = w2
                    nc.scalar.activation(w2[:], t3[:, 0:128], AF.Copy,
                                         scale=r2[:])

                def ph_atbt():
                    inb, e1, w2, t3 = st["inb"], st["e1"], st["w2"], st["t3"]
                    qt = inb[:, QT0:QT0 + LQ]
                    tat = atp.tile([D, LC], F32, tag="at", name=f"at{b}")
                    tbt = btp.tile([D, LC], F32, tag="bt", name=f"bt{b}")
                    st["tat"], st["tbt"] = tat, tbt
                    ob = iop.tile([D, NOUT], F16, tag="ob", name=f"ob{b}")
                    st["ob"] = ob
                    for h in range(2):
                        nc.tensor.matmul(tat[:, 512 * h:512 * (h + 1)],
                                         qt[:],
                                         e1[:, 512 * h:512 * (h + 1)],
                                         start=True, stop=True)
                    for h in range(2):
                        nc.tensor.matmul(tbt[:, 512 * h:512 * (h + 1)],
                                         w2[:],
                                         e1[:, 512 * h:512 * (h + 1)],
                                         start=True, stop=True)
                    for t in range(NT):
                        nc.tensor.matmul(t3[:, 384 + t:385 + t],
                                         e1[:, t * D:(t + 1) * D],
                                         ones1[:], start=True, stop=True)

                def ph_zfin():
                    nc.vector.tensor_copy(st["ob"][:, 2 * LC:2 * LC + NT],
                                          st["t3"][:, 384:392])

                def ph_atcopy():
                    nc.vector.tensor_copy(st["ob"][:, 0:LC], st["tat"][:])

                def ph_btcopy():
                    nc.vector.tensor_copy(st["ob"][:, LC:2 * LC], st["tbt"][:])

                def ph_out():
                    nc.sync.dma_start(Out[b], st["ob"][:])

                st["phases"] = dict(
                    load=ph_load, s0=ph_s0, e2=ph_e2, st=ph_st, e1=ph_e1,
                    w2=ph_w2, atbt=ph_atbt, zfin=ph_zfin, atcopy=ph_atcopy,
                    btcopy=ph_btcopy, out=ph_out)
                return st

            order = [bb for _ in range(reps) for bb in range(NB)]
            N = len(order)
            P = {}
            for k in range(N + 2):
                if 0 <= k - 1 < N:
                    P[k - 1]["phases"]["s0"]()
                    P[k - 1]["phases"]["e2"]()
                if 0 <= k - 2 < N:
                    P[k - 2]["phases"]["atbt"]()
                    P[k - 2]["phases"]["zfin"]()
                    P[k - 2]["phases"]["atcopy"]()
                if 0 <= k - 1 < N:
                    P[k - 1]["phases"]["st"]()
                    P[k - 1]["phases"]["e1"]()
                    P[k - 1]["phases"]["w2"]()
                if 0 <= k - 2 < N:
                    P[k - 2]["phases"]["btcopy"]()
                    P[k - 2]["phases"]["out"]()
                if k < N:
                    P[k] = batch_phases(order[k])
                    P[k]["phases"]["load"]()

    nc.compile()
    return nc


def _prep_inmaps(C, Q, cmask, qmask, w):
    C64 = np.asarray(C, np.float64)
    Q64 = np.asarray(Q, np.float64)
    w64 = np.asarray(w, np.float64)
    wq, wc, wm = w64[:D], w64[D:2 * D], w64[2 * D:]
    C16 = C64.astype(np.float16)                          # [B, D, LC]
    qww16 = (Q64 * wm[None, :, None]).astype(np.float16)  # [B, D, LQ]
    qbias = (1.0 - np.asarray(qmask, np.float64)) * NEG16
    cbias = (1.0 - np.asarray(cmask, np.float64)) * NEG16
    sqf16 = np.ascontiguousarray(
        (np.einsum("bdq,d->bq", Q64, wq) + qbias - LN16)
        .astype(np.float16)[:, :, None])                  # [B, q, 1]
    scb = np.einsum("bdc,d->bc", C64, wc) + cbias         # [B, LC]
    esc = np.exp(scb)                                     # [B, LC]
    ctesc = C64 * esc[:, None, :]                         # [B, D, LC]
    rb = np.empty((B, D, NT, 129), np.float16)
    rb[..., 0:128] = ctesc.reshape(B, D, NT, D).transpose(0, 3, 2, 1)
    rb[..., 128] = esc.reshape(B, NT, D).transpose(0, 2, 1)
    qtb = Q64.astype(np.float16).transpose(0, 2, 1)       # [B, q, d]
    in16 = np.concatenate(
        [C16, qww16, rb.reshape(B, D, NT * 129), qtb, sqf16], axis=2)
    in16 = np.ascontiguousarray(in16)
    in_maps = []
    for k in range(NCORES):
        s = slice(k * NB, (k + 1) * NB)
        in_maps.append({"In16": in16[s]})
    return in_maps


def _run(C, Q, cmask, qmask, w, trace=False):
    from concourse.bass_utils import run_bass_kernel_spmd

    key = (tuple(sorted(CFG.items())), 1)
    if key not in _CACHE:
        _CACHE[key] = _build_nc()
    nc = _CACHE[key]
    in_maps = _prep_inmaps(C, Q, cmask, qmask, w)
    res = run_bass_kernel_spmd(nc, in_maps, core_ids=list(range(NCORES)),
                               trace=trace)
    dev = np.concatenate([res.results[k]["Out"] for k in range(NCORES)],
                         axis=0)                          # [B, D, NOUT]
    return dev, res


def _assemble(dev, C):
    """dev: [n, D, 2*LC+NT] fp16; C: [n, D, LC] input."""
    n = dev.shape[0]
    dv = np.asarray(dev, np.float32)
    z = dv[:, :, 2 * LC:]                                 # [n, j, t] = z1/16
    r1 = (1.0 / z).transpose(0, 2, 1).reshape(n, 1, LC)   # [n, 1, c]
    A = dv[:, :, 0:LC] * r1                               # [n, d, c]
    Bt = dv[:, :, LC:2 * LC] * r1
    Cf = np.asarray(C, np.float32)
    out = np.empty((n, 4 * D, LC), np.float32)
    out[:, 0:D] = Cf
    out[:, D:2 * D] = A
    out[:, 2 * D:3 * D] = Cf * A
    out[:, 3 * D:4 * D] = Cf * Bt
    return out


def kernel(C, Q, cmask, qmask, w):
    dev, _ = _run(C, Q, cmask, qmask, w, trace=False)
    return _assemble(dev, C)


# revision 5
# speedup vs baseline: 2.9028x; 1.9079x over previous
"""CQAttention Trainium2 kernel v3 — [d,c] A/Bt, raw export, host norm.

Device per batch (D=128, Lc=1024, Lq=128):
    e1[q,c]    = exp(s_m^T + s_q - ln16)     ACT, bias = sq32 (bitcast cols)
    e2[c,q]    = exp(s_m)   (8 c-tiles)      ACT
    W2[q,d]    = (e2^T @ [Ct*esc | esc])/z2  8 accum MMs + recip + act
    At[d,c]    = qt^T @ e1                   2 MMs N=512   (= (S1raw/16 @ Qt)^T)
    Bt[d,c]    = w2^T @ e1                   2 MMs N=512
    z[c]=z1/16 = e1_t^T @ 1                  8 tiny MMs (e1-tile stationaries)
Exports fp16 [d, {At 1024 | Bt 1024 | z 8}]; host divides by z (scales
cancel) and assembles [Ct, A, Ct*A, Ct*Bt] in fp32.

PSUM (8 banks): p1 bufs=4 x 1 bank rotating 5 tiles/batch (t2a,t2b = S-tile
quads, t1a,t1b = S^T halves, t3 = W2-accum + z cols); at/bt pools bufs=1 x
2 banks each.
"""

import warnings

warnings.filterwarnings("ignore")

import numpy as np

B, D, LC, LQ = 64, 128, 1024, 128
NT = 8
NCORES = 8
NB = B // NCORES
NEG16 = -30000.0
NIN = LC + LQ + NT * 129 + LQ + 1  # cb | qww | rhsB | qt | sq16
RB0 = LC + LQ
QT0 = RB0 + NT * 129
SQ0 = QT0 + LQ
NOUT = 2 * LC + NT  # At | Bt | z
LN16 = float(np.log(16.0))

CFG = {"v": 4}

_CACHE = {}


def _build_nc(reps=1):
    import concourse.bass as bass
    import concourse.mybir as mybir
    import concourse.tile as tile
    from concourse import bacc

    F32 = mybir.dt.float32
    F16 = mybir.dt.float16
    AF = mybir.ActivationFunctionType

    nc = bacc.Bacc("TRN2", target_bir_lowering=False, debug=False,
                   num_devices=NCORES)

    In16 = nc.dram_tensor("In16", [NB, D, NIN], F16, kind="ExternalInput")
    Out = nc.dram_tensor("Out", [NB, D, NOUT], F16, kind="ExternalOutput")

    with tile.TileContext(nc) as tc:
        with tc.tile_pool(name="const", bufs=1) as constp, \
             tc.tile_pool(name="io", bufs=4) as iop, \
             tc.tile_pool(name="sb", bufs=4) as sb, \
             tc.tile_pool(name="sm", bufs=4) as sm, \
             tc.tile_pool(name="p1", bufs=4, space="PSUM") as p1, \
             tc.tile_pool(name="tab", bufs=1, space="PSUM") as tabp:

            ones1 = constp.tile([D, 1], F16)
            nc.gpsimd.memset(ones1[:], 1.0)

            def batch_phases(b):
                st = {}

                def ph_load():
                    inb = iop.tile([D, NIN], F16, tag="inb", name=f"inb{b}")
                    st["inb"] = inb
                    nc.sync.dma_start(inb[:], In16[b])

                def ph_s0():
                    inb = st["inb"]
                    cb = inb[:, 0:LC]
                    qww = inb[:, LC:LC + LQ]
                    for h in range(2):
                        t2 = p1.tile([D, 4, D], F32, tag="p1",
                                     name=f"t2_{b}_{h}")
                        st[f"t2{h}"] = t2
                        for i in range(4):
                            ct = 4 * h + i
                            nc.tensor.matmul(t2[:, i, :],
                                             cb[:, ct * D:(ct + 1) * D],
                                             qww[:], start=True, stop=True)

                def ph_e2():
                    e2 = sb.tile([D, NT, D], F16, tag="e2", name=f"e2_{b}")
                    st["e2"] = e2
                    for h in range(2):
                        nc.scalar.activation(e2[:, 4 * h:4 * h + 4, :],
                                             st[f"t2{h}"][:], AF.Exp)

                def ph_st():
                    inb = st["inb"]
                    cb = inb[:, 0:LC]
                    qww = inb[:, LC:LC + LQ]
                    for h in range(2):
                        t1 = p1.tile([D, 512], F32, tag="p1",
                                     name=f"t1_{b}_{h}")
                        st[f"t1{h}"] = t1
                        nc.tensor.matmul(t1[:], qww[:],
                                         cb[:, 512 * h:512 * (h + 1)],
                                         start=True, stop=True)

                def ph_e1():
                    inb = st["inb"]
                    sq = inb[:, SQ0:SQ0 + 1]
                    e1 = sb.tile([D, LC], F16, tag="e1", name=f"e1_{b}")
                    st["e1"] = e1
                    for h in range(2):
                        nc.scalar.activation(e1[:, 512 * h:512 * (h + 1)],
                                             st[f"t1{h}"][:], AF.Exp,
                                             bias=sq)

                def ph_w2():
                    inb, e2 = st["inb"], st["e2"]
                    rb = inb[:, RB0:QT0].rearrange("p (t j) -> p t j", t=NT)
                    t3 = p1.tile([D, 512], F32, tag="p1", name=f"t3_{b}")
                    st["t3"] = t3
                    for t in range(NT):
                        nc.tensor.matmul(t3[:, 0:129], e2[:, t, :],
                                         rb[:, t, :],
                                         start=(t == 0), stop=(t == NT - 1))
                    r2 = sm.tile([D, 1], F32, tag="r2", name=f"r2_{b}")
                    nc.vector.reciprocal(r2[:], t3[:, 128:129])
                    w2 = sb.tile([D, D], F16, tag="w2", name=f"w2_{b}")
                    st["w2"] = w2
                    nc.scalar.activation(w2[:], t3[:, 0:128], AF.Copy,
                                         scale=r2[:])

                def ph_atbt():
                    inb, e1, w2, t3 = st["inb"], st["e1"], st["w2"], st["t3"]
                    qt = inb[:, QT0:QT0 + LQ]
                    tab = tabp.tile([D, 2 * LC], F32, tag="tab",
                                    name=f"tab{b}")
                    tat, tbt = tab[:, 0:LC], tab[:, LC:2 * LC]
                    st["tab"] = tab
                    ob = iop.tile([D, NOUT], F16, tag="ob", name=f"ob{b}")
                    st["ob"] = ob
                    for h in range(2):
                        nc.tensor.matmul(tat[:, 512 * h:512 * (h + 1)],
                                         qt[:],
                                         e1[:, 512 * h:512 * (h + 1)],
                                         start=True, stop=True)
                    for h in range(2):
                        nc.tensor.matmul(tbt[:, 512 * h:512 * (h + 1)],
                                         w2[:],
                                         e1[:, 512 * h:512 * (h + 1)],
                                         start=True, stop=True)
                    for t in range(NT):
                        nc.tensor.matmul(t3[:, 384 + t:385 + t],
                                         e1[:, t * D:(t + 1) * D],
                                         ones1[:], start=True, stop=True)

                def ph_zfin():
                    nc.scalar.activation(st["ob"][:, 2 * LC:2 * LC + NT],
                                         st["t3"][:, 384:392], AF.Copy)

                def ph_tabcopy():
                    nc.vector.tensor_copy(st["ob"][:, 0:2 * LC], st["tab"][:])

                def ph_out():
                    nc.sync.dma_start(Out[b], st["ob"][:])

                st["phases"] = dict(
                    load=ph_load, s0=ph_s0, e2=ph_e2, st=ph_st, e1=ph_e1,
                    w2=ph_w2, atbt=ph_atbt, zfin=ph_zfin,
                    tabcopy=ph_tabcopy, out=ph_out)
                return st

            order = [bb for _ in range(reps) for bb in range(NB)]
            N = len(order)
            P = {}
            for k in range(N + 2):
                if 0 <= k - 1 < N:
                    P[k - 1]["phases"]["s0"]()
                    P[k - 1]["phases"]["e2"]()
                if 0 <= k - 2 < N:
                    P[k - 2]["phases"]["atbt"]()
                    P[k - 2]["phases"]["zfin"]()
                    P[k - 2]["phases"]["tabcopy"]()
                if 0 <= k - 1 < N:
                    P[k - 1]["phases"]["st"]()
                    P[k - 1]["phases"]["e1"]()
                    P[k - 1]["phases"]["w2"]()
                if 0 <= k - 2 < N:
                    P[k - 2]["phases"]["out"]()
                if k < N:
                    P[k] = batch_phases(order[k])
                    P[k]["phases"]["load"]()

    nc.compile()
    return nc


def _prep_inmaps(C, Q, cmask, qmask, w):
    C64 = np.asarray(C, np.float64)
    Q64 = np.asarray(Q, np.float64)
    w64 = np.asarray(w, np.float64)
    wq, wc, wm = w64[:D], w64[D:2 * D], w64[2 * D:]
    C16 = C64.astype(np.float16)                          # [B, D, LC]
    qww16 = (Q64 * wm[None, :, None]).astype(np.float16)  # [B, D, LQ]
    qbias = (1.0 - np.asarray(qmask, np.float64)) * NEG16
    cbias = (1.0 - np.asarray(cmask, np.float64)) * NEG16
    sqf16 = np.ascontiguousarray(
        (np.einsum("bdq,d->bq", Q64, wq) + qbias - LN16)
        .astype(np.float16)[:, :, None])                  # [B, q, 1]
    scb = np.einsum("bdc,d->bc", C64, wc) + cbias         # [B, LC]
    esc = np.exp(scb)                                     # [B, LC]
    ctesc = C64 * esc[:, None, :]                         # [B, D, LC]
    rb = np.empty((B, D, NT, 129), np.float16)
    rb[..., 0:128] = ctesc.reshape(B, D, NT, D).transpose(0, 3, 2, 1)
    rb[..., 128] = esc.reshape(B, NT, D).transpose(0, 2, 1)
    qtb = Q64.astype(np.float16).transpose(0, 2, 1)       # [B, q, d]
    in16 = np.concatenate(
        [C16, qww16, rb.reshape(B, D, NT * 129), qtb, sqf16], axis=2)
    in16 = np.ascontiguousarray(in16)
    in_maps = []
    for k in range(NCORES):
        s = slice(k * NB, (k + 1) * NB)
        in_maps.append({"In16": in16[s]})
    return in_maps


def _run(C, Q, cmask, qmask, w, trace=False):
    from concourse.bass_utils import run_bass_kernel_spmd

    key = (tuple(sorted(CFG.items())), 1)
    if key not in _CACHE:
        _CACHE[key] = _build_nc()
    nc = _CACHE[key]
    in_maps = _prep_inmaps(C, Q, cmask, qmask, w)
    res = run_bass_kernel_spmd(nc, in_maps, core_ids=list(range(NCORES)),
                               trace=trace)
    dev = np.concatenate([res.results[k]["Out"] for k in range(NCORES)],
                         axis=0)                          # [B, D, NOUT]
    return dev, res


def _assemble(dev, C):
    """dev: [n, D, 2*LC+NT] fp16; C: [n, D, LC] input."""
    n = dev.shape[0]
    dv = np.asarray(dev, np.float32)
    z = dv[:, :, 2 * LC:]                                 # [n, j, t] = z1/16
    r1 = (1.0 / z).transpose(0, 2, 1).reshape(n, 1, LC)   # [n, 1, c]
    A = dv[:, :, 0:LC] * r1                               # [n, d, c]
    Bt = dv[:, :, LC:2 * LC] * r1
    Cf = np.asarray(C, np.float32)
    out = np.empty((n, 4 * D, LC), np.float32)
    out[:, 0:D] = Cf
    out[:, D:2 * D] = A
    out[:, 2 * D:3 * D] = Cf * A
    out[:, 3 * D:4 * D] = Cf * Bt
    return out


def kernel(C, Q, cmask, qmask, w):
    dev, _ = _run(C, Q, cmask, qmask, w, trace=False)
    return _assemble(dev, C)
